# revision 7
# baseline (speedup 1.0000x reference)
"""Trainium2 Bass kernel for nn_DistanceEnergyModel (gnn_message_passing).

energy = sum over pair/triple edges of MLP(cat(x[edge], sigma)); output is
(energy, d energy/d x).

Strategy (8 NeuronCores, SPMD, data-parallel over edges):
  - Host gathers x[edges] into per-core feature matrices (feature-major:
    [in_dim, E_shard]), casts to bf16.
  - Device runs the fused MLP forward+backward per 512-edge tile:
      z1 = W1^T f          (PE, K=in_dim)
      h1 = silu(z1+b1)     (ACT, silu table)
      t1 = tanh((z1+b1)/2) (ACT, same table set -> no table thrash)
      silu'(z) = 0.5*(1 + t + h*(1-t)) built on DVE from h,t
      z2 = W2^T h1         (PE, K=512)
      h2 = silu(z2+b2) with free-dim accum -> energy partial sums
      gz2 = w3 * silu'(z2) (DVE; 2x factor folded into host-scaled W2T)
      gh1 = (0.5 W2)^T gz2' (PE)
      gz1 = gh1 * silu'(z1) (DVE; 2x folded into host-scaled W1T)
      dfeat = (0.5 W1)^T gz1 (PE) -> DMA out
  - Host: energy = esum . W3 summed over cores; grad via np.bincount
    scatter-add of per-edge dfeat rows.

All matmuls bf16 with fp32 PSUM accumulation.
"""

import numpy as np
import ml_dtypes

import concourse.bass as bass
import concourse.tile as tile
import bass_rust
from concourse import mybir
from concourse.bass_utils import run_bass_kernel_spmd

AF = mybir.ActivationFunctionType
ALU = mybir.AluOpType
F32 = mybir.dt.float32
BF16 = mybir.dt.bfloat16
BF16NP = ml_dtypes.bfloat16

N_CORES = 8
N, DIM = 20000, 2
E2, E3 = 150000, 100000
H = 512
HC = 4          # hidden chunks of 128
P = 128
ET = 512        # edges per tile

E2_LOC = E2 // N_CORES          # 18750
E3_LOC = E3 // N_CORES          # 12500
T_P = -(-E2_LOC // ET)          # 37 tiles
T_T = -(-E3_LOC // ET)          # 25 tiles
E2_PAD = T_P * ET               # 18944
E3_PAD = T_T * ET               # 12800
IN_P, IN_T = 2 * DIM + 2, 3 * DIM + 3   # 6, 9
DOUT_P, DOUT_T = 2 * DIM, 3 * DIM       # 4, 6
ESUM_COLS = (T_P + T_T) * HC            # 248


def _split_waits(nc, max_waits=1):
    """walrus rejects instructions with >1 embedded sync wait; hoist surplus
    waits onto preceding same-engine NOPs (engine streams execute in order)."""
    for f in nc.m.functions:
        for bb in f.blocks:
            new = []
            for inst in bb.instructions:
                si = inst.sync_info
                waits = list(si.on_wait) if (si and si.on_wait) else []
                if len(waits) > max_waits:
                    extra, keep = waits[:-max_waits], waits[-max_waits:]
                    for j, w in enumerate(extra):
                        new.append(mybir.InstNoOp(
                            name=f"{inst.name}-sw{j}",
                            engine=inst.engine, ins=[], outs=[],
                            sync_info=bass_rust.SyncInfo(on_wait=[w], on_update=[]),
                        ))
                    si.on_wait = keep
                new.append(inst)
            bb.instructions = new
    return nc


def _emit_relation(nc, wp, sb, psum, feat_dram, dfeat_dram, esum_sb, esum_col0,
                   w1_sb, w1t_sb, w2_sb, w2t_sb, b1_sb, b1h_sb, b2_sb, b2h_sb,
                   w3_sb, n_tiles, in_dim, dout):
    for t in range(n_tiles):
        ft = sb.tile([in_dim, ET], BF16, tag="feat", name=f"ft_{t}_{in_dim}")
        nc.sync.dma_start(out=ft[:], in_=feat_dram[:, t * ET:(t + 1) * ET])

        h1s, r1s = [], []
        for mc in range(HC):
            z1 = psum.tile([P, ET], F32, tag="z1", name=f"z1_{t}_{mc}_{in_dim}")
            nc.tensor.matmul(out=z1[:], lhsT=w1_sb[:, mc * P:(mc + 1) * P],
                             rhs=ft[:], start=True, stop=True)
            h1 = sb.tile([P, ET], BF16, tag="h1", name=f"h1_{t}_{mc}_{in_dim}")
            nc.scalar.activation(out=h1[:], in_=z1[:], func=AF.Silu,
                                 bias=b1_sb[:, mc:mc + 1])
            t1 = sb.tile([P, ET], BF16, tag="t1", name=f"t1_{t}_{mc}_{in_dim}")
            nc.scalar.activation(out=t1[:], in_=z1[:], func=AF.Tanh,
                                 scale=0.5, bias=b1h_sb[:, mc:mc + 1])
            q1 = sb.tile([P, ET], BF16, tag="q1", name=f"q1_{t}_{mc}_{in_dim}")
            nc.vector.scalar_tensor_tensor(out=q1[:], in0=t1[:], scalar=1.0,
                                           in1=h1[:], op0=ALU.subtract,
                                           op1=ALU.mult)
            r1 = sb.tile([P, ET], BF16, tag="r1", name=f"r1_{t}_{mc}_{in_dim}")
            nc.vector.tensor_tensor(out=r1[:], in0=t1[:], in1=q1[:],
                                    op=ALU.subtract)
            h1s.append(h1)
            r1s.append(r1)

        gz2s = []
        for mc in range(HC):
            z2 = psum.tile([P, ET], F32, tag="z2", name=f"z2_{t}_{mc}_{in_dim}")
            for kc in range(HC):
                nc.tensor.matmul(out=z2[:],
                                 lhsT=w2_sb[:, kc * H + mc * P: kc * H + (mc + 1) * P],
                                 rhs=h1s[kc][:], start=(kc == 0), stop=(kc == HC - 1))
            col = esum_col0 + t * HC + mc
            h2 = sb.tile([P, ET], BF16, tag="h2", name=f"h2_{t}_{mc}_{in_dim}")
            nc.scalar.activation(out=h2[:], in_=z2[:], func=AF.Silu,
                                 bias=b2_sb[:, mc:mc + 1],
                                 accum_out=esum_sb[:, col:col + 1])
            t2 = sb.tile([P, ET], BF16, tag="t2", name=f"t2_{t}_{mc}_{in_dim}")
            nc.scalar.activation(out=t2[:], in_=z2[:], func=AF.Tanh,
                                 scale=0.5, bias=b2h_sb[:, mc:mc + 1])
            q2 = sb.tile([P, ET], BF16, tag="q2", name=f"q2_{t}_{mc}_{in_dim}")
            nc.vector.scalar_tensor_tensor(out=q2[:], in0=t2[:], scalar=1.0,
                                           in1=h2[:], op0=ALU.subtract,
                                           op1=ALU.mult)
            r2 = sb.tile([P, ET], BF16, tag="r2", name=f"r2_{t}_{mc}_{in_dim}")
            nc.vector.tensor_tensor(out=r2[:], in0=t2[:], in1=q2[:],
                                    op=ALU.subtract)
            # gz2' = w3*(1+r2) = 2*w3*silu'(z2); the 0.5 lives in w2t host scale
            gz2 = sb.tile([P, ET], BF16, tag="gz2", name=f"gz2_{t}_{mc}_{in_dim}")
            nc.vector.tensor_scalar(out=gz2[:], in0=r2[:],
                                    scalar1=w3_sb[:, mc:mc + 1],
                                    scalar2=w3_sb[:, mc:mc + 1],
                                    op0=ALU.mult, op1=ALU.add)
            gz2s.append(gz2)

        gz1s = []
        for mc in range(HC):
            gh = psum.tile([P, ET], F32, tag="gh", name=f"gh_{t}_{mc}_{in_dim}")
            for kc in range(HC):
                nc.tensor.matmul(out=gh[:],
                                 lhsT=w2t_sb[:, kc * H + mc * P: kc * H + (mc + 1) * P],
                                 rhs=gz2s[kc][:], start=(kc == 0), stop=(kc == HC - 1))
            # gz1' = (1+r1)*gh = 2*silu'(z1)*gh; the 0.5 lives in w1t host scale
            gz1 = sb.tile([P, ET], BF16, tag="gz1", name=f"gz1_{t}_{mc}_{in_dim}")
            nc.vector.scalar_tensor_tensor(out=gz1[:], in0=r1s[mc][:], scalar=1.0,
                                           in1=gh[:], op0=ALU.add, op1=ALU.mult)
            gz1s.append(gz1)

        df = psum.tile([in_dim, ET], F32, tag="df", name=f"df_{t}_{in_dim}")
        for kc in range(HC):
            nc.tensor.matmul(out=df[:],
                             lhsT=w1t_sb[:, kc * in_dim:(kc + 1) * in_dim],
                             rhs=gz1s[kc][:], start=(kc == 0), stop=(kc == HC - 1))
        dfs = sb.tile([dout, ET], F32, tag="dfs", name=f"dfs_{t}_{in_dim}")
        nc.vector.tensor_copy(out=dfs[:], in_=df[0:dout, :])
        nc.sync.dma_start(out=dfeat_dram[:, t * ET:(t + 1) * ET], in_=dfs[:])


def build_nc():
    nc = bass.Bass()
    dp = nc.declare_dram_parameter

    feat_p = dp("feat_p", [IN_P, E2_PAD], BF16, isOutput=False)
    feat_t = dp("feat_t", [IN_T, E3_PAD], BF16, isOutput=False)
    w1_p = dp("w1_p", [IN_P, H], BF16, isOutput=False)
    w1t_p = dp("w1t_p", [P, HC * IN_P], BF16, isOutput=False)
    w2_p = dp("w2_p", [P, HC * H], BF16, isOutput=False)
    w2t_p = dp("w2t_p", [P, HC * H], BF16, isOutput=False)
    w1_t = dp("w1_t", [IN_T, H], BF16, isOutput=False)
    w1t_t = dp("w1t_t", [P, HC * IN_T], BF16, isOutput=False)
    w2_t = dp("w2_t", [P, HC * H], BF16, isOutput=False)
    w2t_t = dp("w2t_t", [P, HC * H], BF16, isOutput=False)
    bias_p = dp("bias_p", [P, 5 * HC], F32, isOutput=False)  # b1,b1h,b2,b2h,w3
    bias_t = dp("bias_t", [P, 5 * HC], F32, isOutput=False)

    dfeat_p = dp("dfeat_p", [DOUT_P, E2_PAD], F32, isOutput=True)
    dfeat_t = dp("dfeat_t", [DOUT_T, E3_PAD], F32, isOutput=True)
    esum = dp("esum", [P, ESUM_COLS], F32, isOutput=True)

    with tile.TileContext(nc) as tc:
        with (
            tc.tile_pool(name="wp", bufs=1) as wp,
            tc.tile_pool(name="sb", bufs=8) as sb,
            tc.tile_pool(name="ps", bufs=2, space="PSUM") as psum,
        ):
            def load(name, src, shape, dt=BF16):
                tl = wp.tile(shape, dt, name=name)
                nc.sync.dma_start(out=tl[:], in_=src[:, :])
                return tl

            w1p_sb = load("w1p_sb", w1_p, [IN_P, H])
            w1tp_sb = load("w1tp_sb", w1t_p, [P, HC * IN_P])
            w2p_sb = load("w2p_sb", w2_p, [P, HC * H])
            w2tp_sb = load("w2tp_sb", w2t_p, [P, HC * H])
            w1t_sb2 = load("w1t_sb2", w1_t, [IN_T, H])
            w1tt_sb = load("w1tt_sb", w1t_t, [P, HC * IN_T])
            w2t_sb2 = load("w2t_sb2", w2_t, [P, HC * H])
            w2tt_sb = load("w2tt_sb", w2t_t, [P, HC * H])
            bp_sb = load("bp_sb", bias_p, [P, 5 * HC], F32)
            bt_sb = load("bt_sb", bias_t, [P, 5 * HC], F32)

            esum_sb = wp.tile([P, ESUM_COLS], F32, name="esum_sb")

            def bcols(b, k):
                return b[:, k * HC:(k + 1) * HC]

            _emit_relation(nc, wp, sb, psum, feat_p, dfeat_p, esum_sb, 0,
                           w1p_sb, w1tp_sb, w2p_sb, w2tp_sb,
                           bcols(bp_sb, 0), bcols(bp_sb, 1), bcols(bp_sb, 2),
                           bcols(bp_sb, 3), bcols(bp_sb, 4),
                           T_P, IN_P, DOUT_P)
            _emit_relation(nc, wp, sb, psum, feat_t, dfeat_t, esum_sb, T_P * HC,
                           w1t_sb2, w1tt_sb, w2t_sb2, w2tt_sb,
                           bcols(bt_sb, 0), bcols(bt_sb, 1), bcols(bt_sb, 2),
                           bcols(bt_sb, 3), bcols(bt_sb, 4),
                           T_T, IN_T, DOUT_T)

            nc.sync.dma_start(out=esum[:, :], in_=esum_sb[:])

    return _split_waits(nc)


_NC_CACHE = None


def _get_nc():
    global _NC_CACHE
    if _NC_CACHE is None:
        _NC_CACHE = build_nc()
    return _NC_CACHE


_RUNNER_CACHE = None


def _get_runner():
    """Compile once, return run(in_maps)->results closure (reusable for
    timing loops). Mirrors bass2jax.run_bass_via_pjrt's multi-core path but
    caches the jitted executable."""
    global _RUNNER_CACHE
    if _RUNNER_CACHE is not None:
        return _RUNNER_CACHE
    import jax
    from jax.sharding import Mesh, PartitionSpec
    try:
        from jax.experimental.shard_map import shard_map
    except Exception:
        from jax.shard_map import shard_map  # newer jax
    from concourse import bass2jax
    from concourse.bass2jax import (_bass_exec_p, install_neuronx_cc_hook,
                                    partition_id_tensor)

    install_neuronx_cc_hook()
    nc = _get_nc()

    partition_name = (nc.partition_id_tensor.name
                      if nc.partition_id_tensor else None)
    in_names, out_names, out_avals, zero_outs = [], [], [], []
    for alloc in nc.m.functions[0].allocations:
        if not isinstance(alloc, mybir.MemoryLocationSet):
            continue
        name = alloc.memorylocations[0].name
        if alloc.kind == "ExternalInput":
            if name != partition_name:
                in_names.append(name)
        elif alloc.kind == "ExternalOutput":
            out_names.append(name)
            shape = tuple(alloc.tensor_shape)
            dtype = mybir.dt.np(alloc.dtype)
            out_avals.append(jax.core.ShapedArray(shape, dtype))
            zero_outs.append(np.zeros(shape, dtype))
    n_params = len(in_names)
    n_outs = len(out_avals)
    all_in_names = in_names + out_names
    if partition_name is not None:
        all_in_names = all_in_names + [partition_name]
    donate = tuple(range(n_params, n_params + n_outs))

    def _body(*args):
        operands = list(args)
        if partition_name is not None:
            operands.append(partition_id_tensor())
        outs = _bass_exec_p.bind(
            *operands,
            out_avals=tuple(out_avals),
            in_names=tuple(all_in_names),
            out_names=tuple(out_names),
            lowering_input_output_aliases=(),
            sim_require_finite=True,
            sim_require_nnan=True,
            nc=nc,
        )
        return tuple(outs)

    devices = jax.devices()[:N_CORES]
    mesh = Mesh(np.asarray(devices), ("core",))
    sharded = jax.jit(
        shard_map(_body, mesh=mesh,
                  in_specs=(PartitionSpec("core"),) * (n_params + n_outs),
                  out_specs=(PartitionSpec("core"),) * n_outs,
                  check_rep=False),
        donate_argnums=donate, keep_unused=True)

    def run(in_maps, timing_reps=0):
        per_core = [[np.asarray(m[name]) for name in in_names] for m in in_maps]
        concat_in = [np.concatenate([per_core[c][i] for c in range(N_CORES)],
                                    axis=0) for i in range(n_params)]
        concat_zeros = [np.zeros((N_CORES * z.shape[0], *z.shape[1:]), z.dtype)
                        for z in zero_outs]
        out_arrs = sharded(*concat_in, *concat_zeros)
        results = [
            {name: np.asarray(out_arrs[i]).reshape(N_CORES, *out_avals[i].shape)[c]
             for i, name in enumerate(out_names)}
            for c in range(N_CORES)
        ]
        times = []
        if timing_reps:
            import time as _t
            args_in = [jax.device_put(a) for a in concat_in]
            for _ in range(timing_reps):
                zz = [np.zeros((N_CORES * z.shape[0], *z.shape[1:]), z.dtype)
                      for z in zero_outs]
                t0 = _t.perf_counter()
                o = sharded(*args_in, *zz)
                jax.block_until_ready(o)
                times.append(_t.perf_counter() - t0)
        return results, times

    _RUNNER_CACHE = run
    return run


def _pack_weights(W1, b1, W2, b2, W3):
    """Host-side packing for one relation type. Returns dict of arrays."""
    W1 = np.asarray(W1, np.float32)
    W2 = np.asarray(W2, np.float32)
    W3 = np.asarray(W3, np.float32).reshape(-1)
    b1 = np.asarray(b1, np.float32).reshape(-1)
    b2 = np.asarray(b2, np.float32).reshape(-1)
    in_dim = W1.shape[0]
    w2pack = np.ascontiguousarray(
        W2.reshape(HC, P, H).transpose(1, 0, 2).reshape(P, HC * H)).astype(BF16NP)
    w2tpack = np.ascontiguousarray(
        (0.5 * W2.T).reshape(HC, P, H).transpose(1, 0, 2).reshape(P, HC * H)
    ).astype(BF16NP)
    w1tpack = np.ascontiguousarray(
        (0.5 * W1.T).reshape(HC, P, in_dim).transpose(1, 0, 2).reshape(P, HC * in_dim)
    ).astype(BF16NP)
    bias = np.stack([
        b1.reshape(HC, P).T, (0.5 * b1).reshape(HC, P).T,
        b2.reshape(HC, P).T, (0.5 * b2).reshape(HC, P).T,
        W3.reshape(HC, P).T,
    ], axis=1).reshape(P, 5 * HC)  # [P, 5, HC] -> [P, 5*HC]
    bias = np.ascontiguousarray(bias, np.float32)
    return {
        "w1": np.ascontiguousarray(W1).astype(BF16NP),
        "w1t": w1tpack, "w2": w2pack, "w2t": w2tpack, "bias": bias,
    }


def _make_feat(x, edges, sigma, e_loc, e_pad, arity):
    """Feature-major [arity*DIM + arity, e_pad] bf16 feature matrix."""
    in_dim = arity * DIM + arity
    f = np.zeros((in_dim, e_pad), np.float32)
    for a in range(arity):
        f[a * DIM:(a + 1) * DIM, :e_loc] = x[edges[:, a]].T
    f[arity * DIM:, :e_loc] = sigma
    return f.astype(BF16NP)


def _run(inputs, trace=False):
    x = np.asarray(inputs["x"], np.float32)
    sigma = float(np.asarray(inputs["sigma"]).reshape(-1)[0])
    ep = np.asarray(inputs["edges_pair"])
    et = np.asarray(inputs["edges_triple"])
    assert x.shape == (N, DIM) and ep.shape == (E2, 2) and et.shape == (E3, 3)

    packp = _pack_weights(inputs["pW1"], inputs["pb1"], inputs["pW2"],
                          inputs["pb2"], inputs["pW3"])
    packt = _pack_weights(inputs["tW1"], inputs["tb1"], inputs["tW2"],
                          inputs["tb2"], inputs["tW3"])

    common = {
        "w1_p": packp["w1"], "w1t_p": packp["w1t"], "w2_p": packp["w2"],
        "w2t_p": packp["w2t"], "bias_p": packp["bias"],
        "w1_t": packt["w1"], "w1t_t": packt["w1t"], "w2_t": packt["w2"],
        "w2t_t": packt["w2t"], "bias_t": packt["bias"],
    }
    in_maps = []
    for c in range(N_CORES):
        epc = ep[c * E2_LOC:(c + 1) * E2_LOC]
        etc = et[c * E3_LOC:(c + 1) * E3_LOC]
        in_maps.append(dict(
            common,
            feat_p=_make_feat(x, epc, sigma, E2_LOC, E2_PAD, 2),
            feat_t=_make_feat(x, etc, sigma, E3_LOC, E3_PAD, 3),
        ))

    run = _get_runner()
    results, times = run(in_maps, timing_reps=(8 if trace else 0))

    pW3 = np.asarray(inputs["pW3"], np.float32).reshape(-1)
    tW3 = np.asarray(inputs["tW3"], np.float32).reshape(-1)
    pb3 = float(np.asarray(inputs["pb3"]).reshape(-1)[0])
    tb3 = float(np.asarray(inputs["tb3"]).reshape(-1)[0])

    # padded edges have all-zero features; their device energy is the
    # zero-feature edge energy (exactly 0 when biases are zero).  Correct on
    # host with an fp32 recompute of that constant.
    def _e0(W1, b1, W2, b2, W3, b3):
        W1 = np.asarray(W1, np.float32)
        z1 = np.zeros(W1.shape[0], np.float32) @ W1 + np.asarray(b1, np.float32)
        h1 = z1 * (1 / (1 + np.exp(-z1)))
        z2 = h1 @ np.asarray(W2, np.float32) + np.asarray(b2, np.float32)
        h2 = z2 * (1 / (1 + np.exp(-z2)))
        return float(h2 @ np.asarray(W3, np.float32).reshape(-1) + b3)

    e0p = _e0(inputs["pW1"], inputs["pb1"], inputs["pW2"], inputs["pb2"],
              inputs["pW3"], pb3)
    e0t = _e0(inputs["tW1"], inputs["tb1"], inputs["tW2"], inputs["tb2"],
              inputs["tW3"], tb3)

    energy = 0.0
    grad = np.zeros((N, DIM), np.float64)
    for c in range(N_CORES):
        r = results[c]
        es = r["esum"].astype(np.float64)  # [P, ESUM_COLS]
        # feature f = mc*P + p ; column layout: tile-major, chunk minor
        es_p = es[:, :T_P * HC].reshape(P, T_P, HC).sum(1)   # [P, HC]
        es_t = es[:, T_P * HC:].reshape(P, T_T, HC).sum(1)
        h2sum_p = es_p.T.reshape(H)   # feature index mc*P+p
        h2sum_t = es_t.T.reshape(H)
        # S = sum over all device edges (incl. zero-feature pads) of h2.W3;
        # energy = S - n_pad*(e0 - b3) + n_real*b3
        energy += h2sum_p @ pW3 + h2sum_t @ tW3
        energy += E2_LOC * pb3 + E3_LOC * tb3
        energy -= (E2_PAD - E2_LOC) * (e0p - pb3)
        energy -= (E3_PAD - E3_LOC) * (e0t - tb3)

        dfp = r["dfeat_p"][:, :E2_LOC].astype(np.float64)
        dft = r["dfeat_t"][:, :E3_LOC].astype(np.float64)
        epc = ep[c * E2_LOC:(c + 1) * E2_LOC]
        etc = et[c * E3_LOC:(c + 1) * E3_LOC]
        for a in range(2):
            idx = epc[:, a]
            for d in range(DIM):
                grad[:, d] += np.bincount(idx, weights=dfp[a * DIM + d],
                                          minlength=N)
        for a in range(3):
            idx = etc[:, a]
            for d in range(DIM):
                grad[:, d] += np.bincount(idx, weights=dft[a * DIM + d],
                                          minlength=N)

    return (np.float32(energy), grad.astype(np.float32), times)


def kernel(**inputs):
    energy, grad, _ = _run(inputs, trace=False)
    return energy, grad


def kernel_bench(inputs, trace=True):
    energy, grad, times = _run(inputs, trace=trace)
    return energy, grad, times
